# revision 5
# baseline (speedup 1.0000x reference)
"""Trainium2 Bass kernel for nn_DistanceDecoder (GCN stack + per-edge MLPs).

Optimizations over the original one-queue baseline:
  - All dma_gathers alternate the two SWDGE queues (desc-gen parallelism).
  - Propagate gathers use 5-block groups (fewer, bigger gathers; per-gather
    fixed cost ~6-8us dominates small gathers).
  - u1/u2 feature tables are fp8e4 (halves gather bytes + AllGather payload;
    PE matmul takes bf16 one-hot lhsT x fp8 rhs natively).
  - norm = dinv[s]*dinv[d] folded entirely into the one-hot values
    host-side; GCN epilogues lose their per-block scale op.
  - dist pipeline merged into the edge stage: the g table is augmented to
    aug = [g | z] (bf16, 512B rows); one transpose-gather per endpoint
    serves both the MLP features and the distance computation, halving
    edge-stage descriptor count.  The separate z gathers and the
    collective-window interleave are gone.

Harness contract: kernel(**inputs) takes full inputs, returns full [E] f32.
"""

import math
import numpy as np

P = 128
NCORES = 8
ZD = 128
HD = 256
HD2 = HD // 2


# --------------------------------------------------------------------------
# Host-side planning (integer work only: permutation, bucketing, padding)
# --------------------------------------------------------------------------

def build_plan(edge_index, N, ncores=NCORES, gb=None, eb=32):
    src = edge_index[0].astype(np.int64)
    dst = edge_index[1].astype(np.int64)
    E = src.shape[0]

    npc = int(math.ceil(N / ncores / P)) * P      # nodes per core (padded)
    if (npc // P) % 2:
        npc += P                                   # even block count
    npad = npc * ncores
    half = npad // 2
    nb = npc // P                                  # blocks per core
    nblk = npad // P                               # global blocks
    assert half <= 32768, "int16 gather index range exceeded"

    deg = np.bincount(dst, minlength=N).astype(np.int64) + 1   # + self loop

    # stratified permutation: sort by degree desc, deal round-robin over all
    # global blocks; global block i -> (core i % ncores, local block i//ncores)
    order = np.argsort(-deg, kind="stable")
    i = np.arange(N)
    gblk = i % nblk
    slot = i // nblk
    core = gblk % ncores
    lblk = gblk // ncores
    pid = core * npc + lblk * P + slot
    old2new = np.empty(N, np.int64)
    old2new[order] = pid
    new2old = np.full(npad, -1, np.int64)
    new2old[pid] = order

    deg_pad = np.ones(npad, np.float32)
    deg_pad[old2new] = deg.astype(np.float32)

    # ---- scaled-edge lists (E edges + N self loops), bucketed by dst block
    s_all = old2new[np.concatenate([src, np.arange(N)])]
    d_all = old2new[np.concatenate([dst, np.arange(N)])]
    ecore = d_all // npc
    eblk = (d_all % npc) // P
    ecol = d_all % P
    ehalf = (s_all >= half).astype(np.int64)
    srel = s_all - ehalf * half

    # group edges by (core, block, half)
    key = (ecore * nb + eblk) * 2 + ehalf
    ordk = np.argsort(key, kind="stable")
    key_s = key[ordk]
    srel_s = srel[ordk]
    ecol_s = ecol[ordk]
    nkeys = ncores * nb * 2
    counts = np.bincount(key_s, minlength=nkeys)
    cview = counts.reshape(ncores, nb, 2)
    cl_b = tuple(int(math.ceil(cview[:, b, 0].max() / P)) for b in range(nb))
    ch_b = tuple(int(math.ceil(cview[:, b, 1].max() / P)) for b in range(nb))
    cl = sum(cl_b)
    ch = sum(ch_b)
    off_l = np.concatenate([[0], np.cumsum(cl_b)]).astype(int)
    off_h = np.concatenate([[0], np.cumsum(ch_b)]).astype(int)

    # packed per-(core,block,half) chunk arrays (flat, variable per block)
    dinv_pad = deg_pad.astype(np.float64) ** -0.5
    # full GCN normalization dinv[s]*dinv[d] folded into the one-hot values
    dval_all = (dinv_pad[s_all] * dinv_pad[d_all]).astype(np.float32)
    dval_s = dval_all[ordk]
    gidx_lo = np.zeros((ncores, cl * P), np.int16)
    gidx_hi = np.zeros((ncores, ch * P), np.int16)
    dcol_lo = np.full((ncores, cl * P), -1.0, np.float32)
    dcol_hi = np.full((ncores, ch * P), -1.0, np.float32)
    dval_lo = np.zeros((ncores, cl * P), np.float32)
    dval_hi = np.zeros((ncores, ch * P), np.float32)
    starts = np.concatenate([[0], np.cumsum(counts)])
    for c in range(ncores):
        for b in range(nb):
            for h in range(2):
                k = (c * nb + b) * 2 + h
                n = cview[c, b, h]
                sl = slice(starts[k], starts[k] + n)
                if h == 0:
                    base = off_l[b] * P
                    gidx_lo[c, base:base + n] = srel_s[sl]
                    dcol_lo[c, base:base + n] = ecol_s[sl]
                    dval_lo[c, base:base + n] = dval_s[sl]
                else:
                    base = off_h[b] * P
                    gidx_hi[c, base:base + n] = srel_s[sl]
                    dcol_hi[c, base:base + n] = ecol_s[sl]
                    dval_hi[c, base:base + n] = dval_s[sl]

    # wrap indices into the [16, n/16] layout dma_gather expects
    def wrap16(a):
        flat = a.reshape(-1, 16)
        w = np.ascontiguousarray(flat.T).astype(np.int16)
        return np.ascontiguousarray(np.tile(w, (8, 1)))

    def colmajor(a):  # [ncores, C*P] -> per core [P, C]
        out = []
        for c in range(ncores):
            m = a[c].reshape(-1, P)   # [C, P]
            out.append(np.ascontiguousarray(m.T).astype(np.float32))
        return out

    # ---- edge stage: original E edges, round robin over cores, 4 combos
    es = old2new[src]
    ed = old2new[dst]
    ecore2 = np.arange(E) % ncores
    combo = (es >= half).astype(np.int64) * 2 + (ed >= half).astype(np.int64)
    key2 = ecore2 * 4 + combo
    ordk2 = np.argsort(key2, kind="stable")
    counts2 = np.bincount(key2[ordk2], minlength=ncores * 4).reshape(ncores, 4)
    ecs = [max(1, int(math.ceil(counts2[:, k].max() / P))) for k in range(4)]
    nck = sum(ecs)

    eidx_src = np.zeros((ncores, nck * P), np.int16)
    eidx_dst = np.zeros((ncores, nck * P), np.int16)
    slotmap = np.full((ncores, nck * P), -1, np.int64)
    starts2 = np.concatenate([[0], np.cumsum(counts2.reshape(-1))])
    es_rel = (es - (es >= half) * half).astype(np.int16)
    ed_rel = (ed - (ed >= half) * half).astype(np.int16)
    for c in range(ncores):
        off = 0
        for k in range(4):
            kk = c * 4 + k
            n = counts2[c, k]
            sl = ordk2[starts2[kk]:starts2[kk] + n]
            eidx_src[c, off:off + n] = es_rel[sl]
            eidx_dst[c, off:off + n] = ed_rel[sl]
            slotmap[c, off:off + n] = sl
            off += ecs[k] * P

    if gb is None:
        gb = 1
        for g in (5, 2):
            if nb % g == 0:
                gb = g
                break

    meta = dict(npc=npc, npad=npad, half=half, nb=nb, cl=cl, ch=ch,
                cl_b=cl_b, ch_b=ch_b,
                ecs=tuple(ecs), nck=nck, gb=gb, eb=eb, ncores=ncores)
    percore = []
    dcl = colmajor(dcol_lo)
    dch = colmajor(dcol_hi)
    dvl = colmajor(dval_lo)
    dvh = colmajor(dval_hi)
    for c in range(ncores):
        percore.append(dict(
            gidx_lo=wrap16(gidx_lo[c]),
            gidx_hi=wrap16(gidx_hi[c]),
            dcol_lo=dcl[c],
            dcol_hi=dch[c],
            dval_lo=dvl[c],
            dval_hi=dvh[c],
            eidx_src=wrap16(eidx_src[c]),
            eidx_dst=wrap16(eidx_dst[c]),
        ))
    host = dict(old2new=old2new, new2old=new2old, deg_pad=deg_pad,
                slotmap=slotmap)
    return meta, percore, host


# --------------------------------------------------------------------------
# Bass program
# --------------------------------------------------------------------------

def build_nc(meta, debug=False, reps=1, no_coll=False, dbg=False,
             skip=()):
    import concourse.bacc as bacc
    import concourse.tile as tile
    from concourse import mybir

    f32 = mybir.dt.float32
    bf16 = mybir.dt.bfloat16
    fp8 = mybir.dt.float8e4
    i16 = mybir.dt.int16
    AF = mybir.ActivationFunctionType
    OP = mybir.AluOpType

    npc, npad, half = meta["npc"], meta["npad"], meta["half"]
    nb, cl, ch = meta["nb"], meta["cl"], meta["ch"]
    cl_b, ch_b = meta["cl_b"], meta["ch_b"]
    off_l = [0]
    for c_ in cl_b:
        off_l.append(off_l[-1] + c_)
    off_h = [0]
    for c_ in ch_b:
        off_h.append(off_h[-1] + c_)
    ecs, nck = meta["ecs"], meta["nck"]
    gb, eb = meta["gb"], meta["eb"]
    ncores = meta["ncores"]
    zb_gcn, zb_edge, zb_br2 = meta.get("zbias", (False, False, False))
    eb2 = eb
    rg = [list(range(ncores))]

    nc = bacc.Bacc("TRN2", target_bir_lowering=False, debug=debug,
                   num_devices=ncores, num_swdge_queues=2)

    def din(name, shape, dtype):
        return nc.dram_tensor(name, list(shape), dtype, kind="ExternalInput")

    zt_d = din("z_tbl", [npad, ZD], bf16)
    zsh_d = din("z_shard", [npc, ZD], bf16)
    glo_d = din("gidx_lo", [P, cl * 8], i16)
    ghi_d = din("gidx_hi", [P, ch * 8], i16)
    dcl_d = din("dcol_lo", [P, cl], f32)
    dch_d = din("dcol_hi", [P, ch], f32)
    dvl_d = din("dval_lo", [P, cl], f32)
    dvh_d = din("dval_hi", [P, ch], f32)
    esrc_d = din("eidx_src", [P, nck * 8], i16)
    edst_d = din("eidx_dst", [P, nck * 8], i16)
    W0_d = din("W0", [ZD, HD], f32)
    W1_d = din("W1", [HD, HD], f32)
    W2_d = din("W2", [HD, HD], f32)
    W3_d = din("W3", [HD, HD2], f32)
    b0_d = din("b0c", [P, HD], f32)
    b1_d = din("b1c", [P, HD], f32)
    b2_d = din("b2c", [P, HD], f32)
    b3_d = din("b3c", [P, HD2], f32)
    wsrc_d = din("wsrc_cat", [HD2, 2 * HD], f32)
    wdst_d = din("wdst_cat", [HD2, 2 * HD], f32)
    w2q_d = din("w2q", [P, 8], f32)
    brt_d = din("brt_cat", [1, 2 * HD], f32)
    br2_d = din("br2bt2", [P, 2], f32)
    iota_d = din("iota_f", [P, P], f32)
    identf_d = din("ident_f", [P, P], f32)

    out_d = nc.dram_tensor("out", [P, nck], f32, kind="ExternalOutput")
    if dbg:
        dbg_aug_d = nc.dram_tensor("dbg_aug", [1024, 2 * HD2], bf16,
                                   kind="ExternalOutput")
        dbg_u1_d = nc.dram_tensor("dbg_u1", [1024, HD], mybir.dt.float8e4,
                                  kind="ExternalOutput")
        dbg_h1_d = nc.dram_tensor("dbg_h1", [P, HD], f32,
                                  kind="ExternalOutput")
        dbg_h2_d = nc.dram_tensor("dbg_h2", [P, HD], f32,
                                  kind="ExternalOutput")
        dbg_gs_d = nc.dram_tensor("dbg_gs", [P, 2 * 256], f32,
                                  kind="ExternalOutput")
        dbg_gd_d = nc.dram_tensor("dbg_gd", [P, 2 * 256], f32,
                                  kind="ExternalOutput")
        dbg_dif_d = nc.dram_tensor("dbg_dif", [P, 256], f32,
                                   kind="ExternalOutput")
        dbg_nrm_d = nc.dram_tensor("dbg_nrm", [P, 32], f32,
                                   kind="ExternalOutput")
        dbg_acc_d = nc.dram_tensor("dbg_acc", [P, 64], f32,
                                   kind="ExternalOutput")

    from concourse import library_config
    with tile.TileContext(nc) as tc:
        nc.gpsimd.load_library(library_config.mlp)
        with tc.tile_pool(name="dram", bufs=1, space="DRAM") as dram, \
             tc.tile_pool(name="cpool", bufs=1) as cpool, \
             tc.tile_pool(name="spool", bufs=5) as spool, \
             tc.tile_pool(name="dpool", bufs=2) as dpool:

            # alternate the two SWDGE queues across gathers
            qstate = {"q": 0}

            def next_q():
                qstate["q"] ^= 1
                return qstate["q"]

            # ---------- DRAM intermediates ----------
            def alloc_tables():
                return dict(
                    u1_shard=dram.tile([npc, HD], fp8, name="u1_shard"),
                    u1_full=dram.tile([npad, HD], fp8, name="u1_full",
                                      addr_space="Shared"),
                    u2_shard=dram.tile([npc, HD], fp8, name="u2_shard"),
                    u2_full=dram.tile([npad, HD], fp8, name="u2_full",
                                      addr_space="Shared"),
                    t3_shard=dram.tile([npc, HD2], bf16, name="t3_shard"),
                    t3_full=dram.tile([npad, HD2], bf16, name="t3_full",
                                      addr_space="Shared"),
                    aug_shard=dram.tile([npc, 2 * HD2], bf16,
                                        name="aug_shard"),
                    aug_full=dram.tile([npad, 2 * HD2], bf16, name="aug_full",
                                       addr_space="Shared"),
                )

            # ---------- constants into SBUF ----------
            def load_const(dap, shape, dtype, name):
                t = cpool.tile(list(shape), dtype, name=name)
                nc.sync.dma_start(out=t[:], in_=dap)
                return t

            def load_const_bf(dap, shape, name):
                tf = spool.tile(list(shape), f32, name=name + "_f", tag="cvt")
                nc.sync.dma_start(out=tf[:], in_=dap)
                tb = cpool.tile(list(shape), bf16, name=name)
                nc.scalar.copy(out=tb[:], in_=tf[:])
                return tb

            iota_f32 = spool.tile([P, P], f32, name="iota_f32", tag="cvt")
            nc.sync.dma_start(out=iota_f32[:], in_=iota_d.ap())
            iota_sb = cpool.tile([P, P], bf16, name="iota_sb")
            nc.vector.tensor_copy(out=iota_sb[:], in_=iota_f32[:])
            identf_sb = load_const(identf_d.ap(), [P, P], f32, "identf_sb")
            identb_sb = cpool.tile([P, P], bf16, name="identb_sb")
            nc.vector.tensor_copy(out=identb_sb[:], in_=identf_sb[:])
            b0_sb = load_const(b0_d.ap(), [P, HD], f32, "b0_sb")
            b1_sb = load_const(b1_d.ap(), [P, HD], f32, "b1_sb")
            b2_sb = load_const(b2_d.ap(), [P, HD], f32, "b2_sb")
            b3_sb = load_const(b3_d.ap(), [P, HD2], f32, "b3_sb")
            W0_sb = load_const_bf(W0_d.ap(), [ZD, HD], "W0_sb")
            W1a_sb = load_const_bf(W1_d.ap()[0:P, :], [P, HD], "W1a_sb")
            W1b_sb = load_const_bf(W1_d.ap()[P:HD, :], [P, HD], "W1b_sb")
            W2a_sb = load_const_bf(W2_d.ap()[0:P, :], [P, HD], "W2a_sb")
            W2b_sb = load_const_bf(W2_d.ap()[P:HD, :], [P, HD], "W2b_sb")
            W3a_sb = load_const_bf(W3_d.ap()[0:P, :], [P, HD2], "W3a_sb")
            W3b_sb = load_const_bf(W3_d.ap()[P:HD, :], [P, HD2], "W3b_sb")
            wsrc_sb = load_const_bf(wsrc_d.ap(), [HD2, 2 * HD], "wsrc_sb")
            wdst_sb = load_const_bf(wdst_d.ap(), [HD2, 2 * HD], "wdst_sb")
            w2q_sb = load_const_bf(w2q_d.ap(), [P, 8], "w2q_sb")
            brt_sb = load_const_bf(brt_d.ap(), [1, 2 * HD], "brt_sb")
            br2_sb = load_const(br2_d.ap(), [P, 2], f32, "br2_sb")
            ones2_sb = cpool.tile([1, 3 * P], bf16, name="ones2_sb")
            nc.vector.memset(ones2_sb[:], 1.0)
            onesc_sb = cpool.tile([P, 1], bf16, name="onesc_sb")
            nc.vector.memset(onesc_sb[:], 1.0)

            dcl_sb = load_const(dcl_d.ap(), [P, cl], f32, "dcl_sb")
            dch_sb = load_const(dch_d.ap(), [P, ch], f32, "dch_sb")
            dvl_sb = load_const(dvl_d.ap(), [P, cl], f32, "dvl_sb")
            dvh_sb = load_const(dvh_d.ap(), [P, ch], f32, "dvh_sb")
            glo_sb = load_const(glo_d.ap(), [P, cl * 8], i16, "glo_sb")
            ghi_sb = load_const(ghi_d.ap(), [P, ch * 8], i16, "ghi_sb")

            combo_base = [0]
            for k in range(3):
                combo_base.append(combo_base[-1] + ecs[k])

            for _rep in range(reps):
                _t = alloc_tables()
                u1_shard, u1_full = _t["u1_shard"], _t["u1_full"]
                u2_shard, u2_full = _t["u2_shard"], _t["u2_full"]
                t3_shard, t3_full = _t["t3_shard"], _t["t3_full"]
                aug_shard, aug_full = _t["aug_shard"], _t["aug_full"]
                # ---------- GCN phase ----------
                with tc.tile_pool(name="hpool", bufs=2) as hpool, \
                     tc.tile_pool(name="gpool", bufs=3) as gpool, \
                     tc.tile_pool(name="ohpool", bufs=16) as ohpool, \
                     tc.tile_pool(name="ppool", bufs=4, space="PSUM") as ppool, \
                     tc.tile_pool(name="psum", bufs=2, space="PSUM") as psum:

                    # stage the local z rows into aug[:, HD2:] early (pure
                    # DMA; off the critical path).
                    zstage = hpool.tile([P, nb, ZD], bf16, name="zstage",
                                        tag="zst")
                    for b in range(nb):
                        nc.sync.dma_start(
                            out=zstage[:, b, :],
                            in_=zsh_d.ap()[b * P:(b + 1) * P, :])
                        nc.sync.dma_start(
                            out=aug_shard[b * P:(b + 1) * P, HD2:2 * HD2],
                            in_=zstage[:, b, :])

                    h1_sb = hpool.tile([P, nb, HD], bf16, name="h1_sb", tag="h")
                    h2_sb = hpool.tile([P, nb, HD], bf16, name="h2_sb", tag="h")
                    h3_sb = hpool.tile([P, nb, HD], bf16, name="h3_sb", tag="h")

                    def propagate(table, width, dtype, epilogue, gbw=None):
                        gbw = gb if gbw is None else gbw
                        gclmax = max(off_l[b0_ + gbw] - off_l[b0_]
                                     for b0_ in range(0, nb, gbw))
                        gchmax = max(off_h[b0_ + gbw] - off_h[b0_]
                                     for b0_ in range(0, nb, gbw))
                        tlo = table[0:half, :]
                        thi = table[half:npad, :]
                        for grp in range(nb // gbw):
                            b0 = grp * gbw
                            gcl = off_l[b0 + gbw] - off_l[b0]
                            gch = off_h[b0 + gbw] - off_h[b0]
                            glo = gpool.tile([P, gclmax, width], dtype,
                                             name="glo", tag="glo")
                            nc.gpsimd.dma_gather(
                                out_ap=glo[:, 0:gcl, :], in_ap=tlo,
                                idxs_ap=glo_sb[:, off_l[b0] * 8:
                                               (off_l[b0] + gcl) * 8],
                                num_idxs=gcl * P, num_idxs_reg=gcl * P,
                                elem_size=width, single_packet=False,
                                queue_num=next_q())
                            ghi_t = gpool.tile([P, gchmax, width], dtype,
                                               name="ghi_t", tag="ghi")
                            nc.gpsimd.dma_gather(
                                out_ap=ghi_t[:, 0:gch, :], in_ap=thi,
                                idxs_ap=ghi_sb[:, off_h[b0] * 8:
                                               (off_h[b0] + gch) * 8],
                                num_idxs=gch * P, num_idxs_reg=gch * P,
                                elem_size=width, single_packet=False,
                                queue_num=next_q())
                            if "pc" in skip:
                                continue
                            for bb in range(gbw):
                                b = b0 + bb
                                ncl, nch = cl_b[b], ch_b[b]
                                total = ncl + nch
                                ps = ppool.tile([P, width], f32, name="prop_ps",
                                               tag="prop")
                                idx = 0
                                for j in range(ncl):
                                    col = off_l[b] + j
                                    oh = ohpool.tile([P, P], bf16, name="oh",
                                                     tag="oh")
                                    nc.vector.tensor_scalar(
                                        out=oh[:], in0=iota_sb[:],
                                        scalar1=dcl_sb[:, col:col + 1],
                                        scalar2=dvl_sb[:, col:col + 1],
                                        op0=OP.is_equal, op1=OP.mult)
                                    nc.tensor.matmul(
                                        ps[:], lhsT=oh[:],
                                        rhs=glo[:, off_l[b] - off_l[b0] + j, :],
                                        start=(idx == 0),
                                        stop=(idx == total - 1))
                                    idx += 1
                                for j in range(nch):
                                    col = off_h[b] + j
                                    oh = ohpool.tile([P, P], bf16, name="oh",
                                                     tag="oh")
                                    nc.vector.tensor_scalar(
                                        out=oh[:], in0=iota_sb[:],
                                        scalar1=dch_sb[:, col:col + 1],
                                        scalar2=dvh_sb[:, col:col + 1],
                                        op0=OP.is_equal, op1=OP.mult)
                                    nc.tensor.matmul(
                                        ps[:], lhsT=oh[:],
                                        rhs=ghi_t[:, off_h[b] - off_h[b0] + j, :],
                                        start=(idx == 0),
                                        stop=(idx == total - 1))
                                    idx += 1
                                epilogue(b, ps)

                    def transform_block(h_sb, b, wts, outw, dest,
                                        dest_dtype):
                        ups = psum.tile([P, outw], f32, name="ups", tag="mm")
                        nkh = len(wts)
                        for kh in range(nkh):
                            ht_ps = psum.tile([P, P], bf16, name="ht_ps",
                                              tag="tp")
                            nc.tensor.transpose(
                                ht_ps[:], h_sb[:, b, kh * P:(kh + 1) * P],
                                identb_sb[:])
                            ht = spool.tile([P, P], bf16, name="ht", tag="ht")
                            nc.vector.tensor_copy(out=ht[:], in_=ht_ps[:])
                            nc.tensor.matmul(ups[:], lhsT=ht[:],
                                             rhs=wts[kh][:],
                                             start=(kh == 0),
                                             stop=(kh == nkh - 1))
                        usb = spool.tile([P, outw], dest_dtype, name="usb",
                                         tag="usb")
                        nc.scalar.copy(out=usb[:], in_=ups[:])
                        nc.sync.dma_start(out=dest[b * P:(b + 1) * P, :],
                                          in_=usb[:])

                    def epi0(b, ps):
                        s0 = spool.tile([P, ZD], bf16, name="s0", tag="s0")
                        nc.scalar.copy(out=s0[:], in_=ps[:])
                        s0t_ps = psum.tile([P, P], bf16, name="s0t_ps", tag="tp")
                        nc.tensor.transpose(s0t_ps[:], s0[:], identb_sb[:])
                        s0t = spool.tile([P, P], bf16, name="s0t", tag="s0t")
                        nc.vector.tensor_copy(out=s0t[:], in_=s0t_ps[:])
                        hps = psum.tile([P, HD], f32, name="hps", tag="mm")
                        nc.tensor.matmul(hps[:], lhsT=s0t[:], rhs=W0_sb[:],
                                         start=True, stop=True)
                        if not zb_gcn:
                            nc.vector.tensor_tensor(out=hps[:], in0=hps[:],
                                                    in1=b0_sb[:], op=OP.add)
                        nc.scalar.activation(h1_sb[:, b, :], hps[:], AF.Relu)
                        transform_block(h1_sb, b, [W1a_sb, W1b_sb], HD,
                                        u1_shard, fp8)

                    if "gcn" not in skip:
                        propagate(zt_d.ap(), ZD, bf16, epi0)


                    def epi_mid(h_next, bias_sb, twts, toutw, tdest,
                                tdtype):
                        def epi(b, ps):
                            if not zb_gcn:
                                nc.vector.tensor_tensor(out=ps[:], in0=ps[:],
                                                        in1=bias_sb[:],
                                                        op=OP.add)
                            nc.scalar.activation(h_next[:, b, :], ps[:], AF.Relu)
                            transform_block(h_next, b, twts, toutw, tdest,
                                            tdtype)
                        return epi

                    def do_coll(shard, full):
                        if no_coll:
                            nc.sync.dma_start(out=full[0:npc, :], in_=shard[:])
                        else:
                            nc.gpsimd.collective_compute(
                                "AllGather", OP.bypass, replica_groups=rg,
                                ins=[shard[:].opt()], outs=[full[:].opt()])

                    if "gcn" not in skip:
                        do_coll(u1_shard, u1_full)
                        propagate(u1_full, HD, fp8,
                                  epi_mid(h2_sb, b1_sb, [W2a_sb, W2b_sb], HD,
                                          u2_shard, fp8))

                        do_coll(u2_shard, u2_full)
                        propagate(u2_full, HD, fp8,
                                  epi_mid(h3_sb, b2_sb, [W3a_sb, W3b_sb],
                                          HD2, t3_shard, bf16))

                        do_coll(t3_shard, t3_full)

                    def epi3(b, ps):
                        if not zb_gcn:
                            nc.vector.tensor_tensor(out=ps[:], in0=ps[:],
                                                    in1=b3_sb[:], op=OP.add)
                        gb_ = spool.tile([P, HD2], bf16, name="gb_", tag="gb_")
                        nc.scalar.copy(out=gb_[:], in_=ps[:])
                        nc.sync.dma_start(
                            out=aug_shard[b * P:(b + 1) * P, 0:HD2],
                            in_=gb_[:])

                    if "gcn" not in skip:
                        propagate(t3_full, HD2, bf16, epi3)
                    do_coll(aug_shard, aug_full)
                    if dbg and _rep == 0:
                        nc.sync.dma_start(out=dbg_aug_d.ap(),
                                          in_=aug_full[0:1024, :])
                        nc.sync.dma_start(out=dbg_u1_d.ap(),
                                          in_=u1_full[0:1024, :])
                        dh1 = spool.tile([P, HD], f32, name="dh1", tag="cvt")
                        nc.vector.tensor_copy(out=dh1[:], in_=h1_sb[:, 0, :])
                        nc.sync.dma_start(out=dbg_h1_d.ap(), in_=dh1[:])
                        dh2 = spool.tile([P, HD], f32, name="dh2", tag="cvt")
                        nc.vector.tensor_copy(out=dh2[:], in_=h2_sb[:, 0, :])
                        nc.sync.dma_start(out=dbg_h2_d.ap(), in_=dh2[:])

                if "edge" in skip:
                    nc.sync.dma_start(
                        out=out_d.ap()[:, :],
                        in_=glo_sb[:, 0:2 * nck].bitcast(f32))
                    continue
                # ---------- edge stage ----------
                with tc.tile_pool(name="epool", bufs=3) as epool, \
                     tc.tile_pool(name="fpool", bufs=3) as fpool, \
                     tc.tile_pool(name="jpool", bufs=6) as jpool, \
                     tc.tile_pool(name="idxp", bufs=1) as idxp, \
                     tc.tile_pool(name="hps_pool", bufs=2, space="PSUM") as hps_pool, \
                     tc.tile_pool(name="dps", bufs=1, space="PSUM") as dps, \
                     tc.tile_pool(name="tpp", bufs=2, space="PSUM") as tpp, \
                     tc.tile_pool(name="rt_pool", bufs=1, space="PSUM") as rt_pool:

                    esrc_sb = idxp.tile([P, nck * 8], i16, name="esrc_sb")
                    nc.sync.dma_start(out=esrc_sb[:], in_=esrc_d.ap())
                    edst_sb = idxp.tile([P, nck * 8], i16, name="edst_sb")
                    nc.sync.dma_start(out=edst_sb[:], in_=edst_d.ap())

                    alo_t = aug_full[0:half, :]
                    ahi_t = aug_full[half:npad, :]

                    for k in range(4):
                        s_a = ahi_t if k >= 2 else alo_t
                        d_a = ahi_t if (k % 2) else alo_t
                        nchunks = ecs[k]
                        for c0 in range(0, nchunks, eb2):
                            nbch = min(eb2, nchunks - c0)
                            base = combo_base[k] + c0
                            idx_s = esrc_sb[:, base * 8:(base + nbch) * 8]
                            idx_d = edst_sb[:, base * 8:(base + nbch) * 8]

                            def egather(tab, idxs, name):
                                t = epool.tile([P, 2, nbch * P], bf16,
                                               name=name, tag=name)
                                nc.gpsimd.dma_gather(
                                    out_ap=t[:], in_ap=tab,
                                    idxs_ap=idxs,
                                    num_idxs=nbch * P, num_idxs_reg=nbch * P,
                                    elem_size=2 * HD2, transpose=True,
                                    single_packet=False, queue_num=0)
                                return t

                            gs = egather(s_a, idx_s, "gs")
                            # dst endpoint: edge-major full-row gather on q1
                            # (non-transpose; safe concurrently with the q0
                            # transpose gather), PE-transposed per chunk.
                            gdr = epool.tile([P, nbch, 2 * HD2], bf16,
                                             name="gdr", tag="gdr")
                            nc.gpsimd.dma_gather(
                                out_ap=gdr[:], in_ap=d_a,
                                idxs_ap=idx_d,
                                num_idxs=nbch * P, num_idxs_reg=nbch * P,
                                elem_size=2 * HD2, single_packet=False,
                                queue_num=1)

                            ps_d = dps.tile([P, eb2], f32, name="ps_d",
                                            tag="ps_d")
                            ps_rt = rt_pool.tile([P, eb2, 2], f32, name="ps_rt",
                                                 tag="ps_rt")
                            for cc0 in range(0, nbch, 2):
                                w = min(2, nbch - cc0)
                                cols = w * P
                                # transpose dst g|z halves to [feat, edge]
                                gdt2 = jpool.tile([P, 2, 2 * P], bf16,
                                                  name="gdt2", tag="gdt2")
                                for i in range(w):
                                    for hh in range(2):
                                        etp = tpp.tile([P, P], bf16,
                                                       name="etp", tag="etp")
                                        nc.tensor.transpose(
                                            etp[:],
                                            gdr[:, cc0 + i,
                                                hh * P:(hh + 1) * P],
                                            identb_sb[:])
                                        nc.vector.tensor_copy(
                                            out=gdt2[:, hh, i * P:(i + 1) * P],
                                            in_=etp[:])
                                diffp = fpool.tile([P, 2 * P], bf16,
                                                   name="diffp", tag="diffp")
                                nc.vector.tensor_tensor(
                                    out=diffp[:, 0:cols],
                                    in0=gs[:, 1, cc0 * P:cc0 * P + cols],
                                    in1=gdt2[:, 1, 0:cols],
                                    op=OP.subtract)
                                sqp = fpool.tile([P, 2 * P], bf16, name="sqp",
                                                 tag="sqp")
                                nc.vector.tensor_tensor(
                                    out=sqp[:, 0:cols], in0=diffp[:, 0:cols],
                                    in1=diffp[:, 0:cols], op=OP.mult)
                                for i in range(w):
                                    cc = cc0 + i
                                    nc.tensor.matmul(
                                        ps_d[:, cc:cc + 1],
                                        lhsT=sqp[:, i * P:(i + 1) * P],
                                        rhs=onesc_sb[:], start=True, stop=True)
                                ps_h = hps_pool.tile([P, 4, 2 * P], f32,
                                                     name="ps_h", tag="ps_h")
                                for q in range(4):
                                    if not zb_edge:
                                        nc.tensor.matmul(
                                            ps_h[:, q, 0:cols],
                                            lhsT=brt_sb[:, q * P:(q + 1) * P],
                                            rhs=ones2_sb[:, 0:cols],
                                            start=True, stop=False)
                                    nc.tensor.matmul(
                                        ps_h[:, q, 0:cols],
                                        lhsT=wsrc_sb[:, q * P:(q + 1) * P],
                                        rhs=gs[:, 0, cc0 * P:cc0 * P + cols],
                                        start=zb_edge, stop=False)
                                    nc.tensor.matmul(
                                        ps_h[:, q, 0:cols],
                                        lhsT=wdst_sb[:, q * P:(q + 1) * P],
                                        rhs=gdt2[:, 0, 0:cols],
                                        start=False, stop=True)
                                hact = jpool.tile([P, 4, 2 * P], bf16,
                                                  name="hact", tag="hact")
                                nc.scalar.activation(hact[:, :, 0:cols],
                                                     ps_h[:, :, 0:cols],
                                                     AF.Lrelu, alpha=0.2)
                                for i in range(w):
                                    cc = cc0 + i
                                    for q in range(4):
                                        nc.tensor.matmul(
                                            ps_rt[:, cc, 0:2],
                                            lhsT=hact[:, q, i * P:(i + 1) * P],
                                            rhs=w2q_sb[:, 2 * q:2 * q + 2],
                                            start=(q == 0), stop=(q == 3))
                            nrmb = fpool.tile([P, eb2], f32,
                                              name="nrmb", tag="nrmb")
                            nc.scalar.sqrt(out=nrmb[:, 0:nbch],
                                           in_=ps_d[:, 0:nbch])
                            # finalize batch
                            acc = fpool.tile([P, eb2, 2], f32, name="acc",
                                             tag="acc")
                            nc.vector.tensor_copy(out=acc[:, 0:nbch, :],
                                                  in_=ps_rt[:, 0:nbch, :])
                            tt = fpool.tile([P, eb2], f32, name="tt", tag="tt")
                            nc.vector.tensor_scalar(
                                out=tt[:, 0:nbch],
                                in0=acc[:, 0:nbch, 1],
                                scalar1=br2_sb[:, 1:2], scalar2=None,
                                op0=OP.add)
                            tinv = fpool.tile([P, eb2], f32, name="tinv",
                                              tag="tinv")
                            nc.vector.reciprocal(out=tinv[:, 0:nbch],
                                                 in_=tt[:, 0:nbch])
                            num = fpool.tile([P, eb2], f32, name="num", tag="num")
                            nc.vector.tensor_tensor(out=num[:, 0:nbch],
                                                    in0=nrmb[:, 0:nbch],
                                                    in1=acc[:, 0:nbch, 0],
                                                    op=OP.add)
                            if not zb_br2:
                                nc.vector.tensor_scalar(
                                    out=num[:, 0:nbch], in0=num[:, 0:nbch],
                                    scalar1=br2_sb[:, 0:1], scalar2=None,
                                    op0=OP.add)
                            xx = fpool.tile([P, eb2], f32, name="xx", tag="xx")
                            nc.vector.tensor_tensor(out=xx[:, 0:nbch],
                                                    in0=num[:, 0:nbch],
                                                    in1=tinv[:, 0:nbch],
                                                    op=OP.mult)
                            th = fpool.tile([P, eb2], f32, name="th", tag="th")
                            nc.scalar.activation(th[:, 0:nbch], xx[:, 0:nbch],
                                                 AF.Tanh, scale=-0.5)
                            osb = fpool.tile([P, eb2], f32, name="osb", tag="osb")
                            nc.vector.tensor_scalar(
                                out=osb[:, 0:nbch], in0=th[:, 0:nbch],
                                scalar1=0.5, scalar2=0.5,
                                op0=OP.mult, op1=OP.add)
                            nc.sync.dma_start(
                                out=out_d.ap()[:, base:base + nbch],
                                in_=osb[:, 0:nbch])
    nc.finalize()
    return nc


# --------------------------------------------------------------------------
# Input staging
# --------------------------------------------------------------------------

def stage_inputs(meta, percore, host, inputs):
    npc, nb = meta["npc"], meta["nb"]
    ncores = meta["ncores"]
    old2new = host["old2new"]
    z = np.asarray(inputs["z"], np.float32)

    zpad = np.zeros((meta["npad"], ZD), np.float32)
    zpad[old2new] = z
    import ml_dtypes
    zt = np.ascontiguousarray(zpad.astype(ml_dtypes.bfloat16))

    def bc(v, w):
        v = np.asarray(v, np.float32).reshape(-1)
        return np.ascontiguousarray(np.broadcast_to(v, (P, w)))

    Wr1 = np.asarray(inputs["Wr1"], np.float32)
    Wt1 = np.asarray(inputs["Wt1"], np.float32)
    wsrc = np.ascontiguousarray(
        np.concatenate([Wr1[:HD2], Wt1[:HD2]], axis=1))
    wdst = np.ascontiguousarray(
        np.concatenate([Wr1[HD2:], Wt1[HD2:]], axis=1))
    wr2 = np.asarray(inputs["Wr2"], np.float32)[:, 0]
    wt2 = np.asarray(inputs["Wt2"], np.float32)[:, 0]
    w2q = np.zeros((P, 4, 2), np.float32)
    w2q[:, 0, 0] = wr2[0:P]
    w2q[:, 1, 0] = wr2[P:HD]
    w2q[:, 2, 1] = wt2[0:P]
    w2q[:, 3, 1] = wt2[P:HD]
    brt = np.ascontiguousarray(np.concatenate(
        [np.asarray(inputs["br1"], np.float32),
         np.asarray(inputs["bt1"], np.float32)])[None, :])
    br2v = np.array([[float(np.asarray(inputs["br2"]).reshape(-1)[0]),
                      float(np.asarray(inputs["bt2"]).reshape(-1)[0])]],
                    np.float32)
    iota = np.ascontiguousarray(
        np.broadcast_to(np.arange(P, dtype=np.float32)[None, :], (P, P)))

    in_maps = []
    for c in range(ncores):
        pc = percore[c]
        m = {
            "z_tbl": zt,
            "z_shard": np.ascontiguousarray(zt[c * npc:(c + 1) * npc]),
            "gidx_lo": pc["gidx_lo"], "gidx_hi": pc["gidx_hi"],
            "dcol_lo": pc["dcol_lo"], "dcol_hi": pc["dcol_hi"],
            "dval_lo": pc["dval_lo"], "dval_hi": pc["dval_hi"],
            "eidx_src": pc["eidx_src"], "eidx_dst": pc["eidx_dst"],
            "W0": np.asarray(inputs["W0"], np.float32),
            "W1": np.asarray(inputs["W1"], np.float32),
            "W2": np.asarray(inputs["W2"], np.float32),
            "W3": np.asarray(inputs["W3"], np.float32),
            "b0c": bc(inputs["b0"], HD), "b1c": bc(inputs["b1"], HD),
            "b2c": bc(inputs["b2"], HD), "b3c": bc(inputs["b3"], HD2),
            "wsrc_cat": wsrc, "wdst_cat": wdst,
            "w2q": np.ascontiguousarray(w2q.reshape(P, 8)),
            "brt_cat": brt,
            "br2bt2": np.ascontiguousarray(np.broadcast_to(br2v, (P, 2))),
            "iota_f": iota,
            "ident_f": np.eye(P, dtype=np.float32),
        }
        in_maps.append(m)
    return in_maps


def assemble_output(meta, host, results, E):
    out = np.zeros(E, np.float32)
    slotmap = host["slotmap"]
    for c in range(meta["ncores"]):
        buf = np.asarray(results[c]["out"]).astype(np.float32)  # [P, nck]
        vals = buf.T.reshape(-1)                   # slot = chunk*P + p
        sm = slotmap[c]
        ok = sm >= 0
        out[sm[ok]] = vals[ok]
    return out


# --------------------------------------------------------------------------
# Entry point
# --------------------------------------------------------------------------

_CACHE = {}


def kernel(**inputs):
    edge_index = np.asarray(inputs["edge_index"])
    N = np.asarray(inputs["z"]).shape[0]
    E = edge_index.shape[1]

    meta, percore, host = build_plan(edge_index, N)
    zb_gcn = all(
        not np.any(np.asarray(inputs[k])) for k in ("b0", "b1", "b2", "b3"))
    zb_edge = not (np.any(np.asarray(inputs["br1"]))
                   or np.any(np.asarray(inputs["bt1"])))
    zb_br2 = not np.any(np.asarray(inputs["br2"]))
    meta["zbias"] = (zb_gcn, zb_edge, zb_br2)
    key = tuple(sorted((k, v) for k, v in meta.items()))
    if key not in _CACHE:
        _CACHE[key] = build_nc(meta, debug=False)
    nc = _CACHE[key]

    in_maps = stage_inputs(meta, percore, host, inputs)
    from concourse.bass_utils import run_bass_kernel_spmd
    import os
    trace = bool(int(os.environ.get("KERNEL_TRACE", "0")))
    res = run_bass_kernel_spmd(nc, in_maps,
                               core_ids=list(range(meta["ncores"])),
                               trace=trace)
    kernel._last_res = res
    return assemble_output(meta, host, res.results, E)


# revision 6
# speedup vs baseline: 1.2540x; 1.2540x over previous
"""Trainium2 Bass kernel for nn_DistanceDecoder (GCN stack + per-edge MLPs).

Optimizations over the original one-queue baseline (~2x kernel-body):
  - GCN propagate gathers (non-transpose) alternate the two SWDGE queues
    and use 5-block groups: desc-gen parallelism + amortized ~6-8us
    per-gather fixed cost.  NOTE: transpose-mode gathers must NOT run
    concurrently on different queues -- a ucode bug cross-contaminates
    their descriptor streams (verified empirically; index channels of one
    gather read another gather's index tile).
  - u1/u2 feature tables are fp8e4 (halves gather bytes + AllGather
    payload; PE matmul takes bf16 one-hot lhsT x fp8 rhs natively,
    verified exact).  End-to-end error stays ~1.3e-4 (budget 2e-2).
  - norm = dinv[s]*dinv[d] folded entirely into the one-hot values
    host-side; GCN epilogues lose their per-block scale op.
  - Each layer's transform (transpose + weight matmul + table write) is
    interleaved into the propagate epilogue of the block that produced it,
    so the AllGather starts as soon as the last block lands.
  - dist pipeline merged into the edge stage via the augmented table
    aug = [g | z] (bf16, 512B rows).  Per edge batch: the src endpoint is
    one elem-256 transpose gather on q0; the dst endpoint is a
    non-transpose full-row gather on q1 (safe concurrently with the q0
    transpose gather) whose g|z halves are PE-transposed per chunk.  This
    splits the gather load across both queues; edge pools are 3-deep so
    batch i+2's gathers overlap batch i's compute.

Harness contract: kernel(**inputs) takes full inputs, returns full [E] f32.
"""

import math
import numpy as np

P = 128
NCORES = 8
ZD = 128
HD = 256
HD2 = HD // 2


# --------------------------------------------------------------------------
# Host-side planning (integer work only: permutation, bucketing, padding)
# --------------------------------------------------------------------------

def build_plan(edge_index, N, ncores=NCORES, gb=None, eb=32):
    src = edge_index[0].astype(np.int64)
    dst = edge_index[1].astype(np.int64)
    E = src.shape[0]

    npc = int(math.ceil(N / ncores / P)) * P      # nodes per core (padded)
    if (npc // P) % 2:
        npc += P                                   # even block count
    npad = npc * ncores
    half = npad // 2
    nb = npc // P                                  # blocks per core
    nblk = npad // P                               # global blocks
    assert half <= 32768, "int16 gather index range exceeded"

    deg = np.bincount(dst, minlength=N).astype(np.int64) + 1   # + self loop

    # stratified permutation: sort by degree desc, deal round-robin over all
    # global blocks; global block i -> (core i % ncores, local block i//ncores)
    order = np.argsort(-deg, kind="stable")
    i = np.arange(N)
    gblk = i % nblk
    slot = i // nblk
    core = gblk % ncores
    lblk = gblk // ncores
    pid = core * npc + lblk * P + slot
    old2new = np.empty(N, np.int64)
    old2new[order] = pid
    new2old = np.full(npad, -1, np.int64)
    new2old[pid] = order

    deg_pad = np.ones(npad, np.float32)
    deg_pad[old2new] = deg.astype(np.float32)

    # ---- scaled-edge lists (E edges + N self loops), bucketed by dst block
    s_all = old2new[np.concatenate([src, np.arange(N)])]
    d_all = old2new[np.concatenate([dst, np.arange(N)])]
    ecore = d_all // npc
    eblk = (d_all % npc) // P
    ecol = d_all % P
    ehalf = (s_all >= half).astype(np.int64)
    srel = s_all - ehalf * half

    # group edges by (core, block, half)
    key = (ecore * nb + eblk) * 2 + ehalf
    ordk = np.argsort(key, kind="stable")
    key_s = key[ordk]
    srel_s = srel[ordk]
    ecol_s = ecol[ordk]
    nkeys = ncores * nb * 2
    counts = np.bincount(key_s, minlength=nkeys)
    cview = counts.reshape(ncores, nb, 2)
    cl_b = tuple(int(math.ceil(cview[:, b, 0].max() / P)) for b in range(nb))
    ch_b = tuple(int(math.ceil(cview[:, b, 1].max() / P)) for b in range(nb))
    cl = sum(cl_b)
    ch = sum(ch_b)
    off_l = np.concatenate([[0], np.cumsum(cl_b)]).astype(int)
    off_h = np.concatenate([[0], np.cumsum(ch_b)]).astype(int)

    # packed per-(core,block,half) chunk arrays (flat, variable per block)
    dinv_pad = deg_pad.astype(np.float64) ** -0.5
    # full GCN normalization dinv[s]*dinv[d] folded into the one-hot values
    dval_all = (dinv_pad[s_all] * dinv_pad[d_all]).astype(np.float32)
    dval_s = dval_all[ordk]
    gidx_lo = np.zeros((ncores, cl * P), np.int16)
    gidx_hi = np.zeros((ncores, ch * P), np.int16)
    dcol_lo = np.full((ncores, cl * P), -1.0, np.float32)
    dcol_hi = np.full((ncores, ch * P), -1.0, np.float32)
    dval_lo = np.zeros((ncores, cl * P), np.float32)
    dval_hi = np.zeros((ncores, ch * P), np.float32)
    starts = np.concatenate([[0], np.cumsum(counts)])
    for c in range(ncores):
        for b in range(nb):
            for h in range(2):
                k = (c * nb + b) * 2 + h
                n = cview[c, b, h]
                sl = slice(starts[k], starts[k] + n)
                if h == 0:
                    base = off_l[b] * P
                    gidx_lo[c, base:base + n] = srel_s[sl]
                    dcol_lo[c, base:base + n] = ecol_s[sl]
                    dval_lo[c, base:base + n] = dval_s[sl]
                else:
                    base = off_h[b] * P
                    gidx_hi[c, base:base + n] = srel_s[sl]
                    dcol_hi[c, base:base + n] = ecol_s[sl]
                    dval_hi[c, base:base + n] = dval_s[sl]

    # wrap indices into the [16, n/16] layout dma_gather expects
    def wrap16(a):
        flat = a.reshape(-1, 16)
        w = np.ascontiguousarray(flat.T).astype(np.int16)
        return np.ascontiguousarray(np.tile(w, (8, 1)))

    def colmajor(a):  # [ncores, C*P] -> per core [P, C]
        out = []
        for c in range(ncores):
            m = a[c].reshape(-1, P)   # [C, P]
            out.append(np.ascontiguousarray(m.T).astype(np.float32))
        return out

    # ---- edge stage: original E edges, round robin over cores, 4 combos
    es = old2new[src]
    ed = old2new[dst]
    ecore2 = np.arange(E) % ncores
    combo = (es >= half).astype(np.int64) * 2 + (ed >= half).astype(np.int64)
    key2 = ecore2 * 4 + combo
    ordk2 = np.argsort(key2, kind="stable")
    counts2 = np.bincount(key2[ordk2], minlength=ncores * 4).reshape(ncores, 4)
    ecs = [max(1, int(math.ceil(counts2[:, k].max() / P))) for k in range(4)]
    nck = sum(ecs)

    eidx_src = np.zeros((ncores, nck * P), np.int16)
    eidx_dst = np.zeros((ncores, nck * P), np.int16)
    slotmap = np.full((ncores, nck * P), -1, np.int64)
    starts2 = np.concatenate([[0], np.cumsum(counts2.reshape(-1))])
    es_rel = (es - (es >= half) * half).astype(np.int16)
    ed_rel = (ed - (ed >= half) * half).astype(np.int16)
    for c in range(ncores):
        off = 0
        for k in range(4):
            kk = c * 4 + k
            n = counts2[c, k]
            sl = ordk2[starts2[kk]:starts2[kk] + n]
            eidx_src[c, off:off + n] = es_rel[sl]
            eidx_dst[c, off:off + n] = ed_rel[sl]
            slotmap[c, off:off + n] = sl
            off += ecs[k] * P

    if gb is None:
        gb = 1
        for g in (5, 2):
            if nb % g == 0:
                gb = g
                break

    meta = dict(npc=npc, npad=npad, half=half, nb=nb, cl=cl, ch=ch,
                cl_b=cl_b, ch_b=ch_b,
                ecs=tuple(ecs), nck=nck, gb=gb, eb=eb, ncores=ncores)
    percore = []
    dcl = colmajor(dcol_lo)
    dch = colmajor(dcol_hi)
    dvl = colmajor(dval_lo)
    dvh = colmajor(dval_hi)
    for c in range(ncores):
        percore.append(dict(
            gidx_lo=wrap16(gidx_lo[c]),
            gidx_hi=wrap16(gidx_hi[c]),
            dcol_lo=dcl[c],
            dcol_hi=dch[c],
            dval_lo=dvl[c],
            dval_hi=dvh[c],
            eidx_src=wrap16(eidx_src[c]),
            eidx_dst=wrap16(eidx_dst[c]),
        ))
    host = dict(old2new=old2new, new2old=new2old, deg_pad=deg_pad,
                slotmap=slotmap)
    return meta, percore, host


# --------------------------------------------------------------------------
# Bass program
# --------------------------------------------------------------------------

def build_nc(meta, debug=False, reps=1, no_coll=False, dbg=False,
             skip=()):
    import concourse.bacc as bacc
    import concourse.tile as tile
    from concourse import mybir

    f32 = mybir.dt.float32
    bf16 = mybir.dt.bfloat16
    fp8 = mybir.dt.float8e4
    i16 = mybir.dt.int16
    AF = mybir.ActivationFunctionType
    OP = mybir.AluOpType

    npc, npad, half = meta["npc"], meta["npad"], meta["half"]
    nb, cl, ch = meta["nb"], meta["cl"], meta["ch"]
    cl_b, ch_b = meta["cl_b"], meta["ch_b"]
    off_l = [0]
    for c_ in cl_b:
        off_l.append(off_l[-1] + c_)
    off_h = [0]
    for c_ in ch_b:
        off_h.append(off_h[-1] + c_)
    ecs, nck = meta["ecs"], meta["nck"]
    gb, eb = meta["gb"], meta["eb"]
    ncores = meta["ncores"]
    zb_gcn, zb_edge, zb_br2 = meta.get("zbias", (False, False, False))
    eb2 = eb
    rg = [list(range(ncores))]

    nc = bacc.Bacc("TRN2", target_bir_lowering=False, debug=debug,
                   num_devices=ncores, num_swdge_queues=2)

    def din(name, shape, dtype):
        return nc.dram_tensor(name, list(shape), dtype, kind="ExternalInput")

    zt_d = din("z_tbl", [npad, ZD], bf16)
    zsh_d = din("z_shard", [npc, ZD], bf16)
    glo_d = din("gidx_lo", [P, cl * 8], i16)
    ghi_d = din("gidx_hi", [P, ch * 8], i16)
    dcl_d = din("dcol_lo", [P, cl], f32)
    dch_d = din("dcol_hi", [P, ch], f32)
    dvl_d = din("dval_lo", [P, cl], f32)
    dvh_d = din("dval_hi", [P, ch], f32)
    esrc_d = din("eidx_src", [P, nck * 8], i16)
    edst_d = din("eidx_dst", [P, nck * 8], i16)
    W0_d = din("W0", [ZD, HD], f32)
    W1_d = din("W1", [HD, HD], f32)
    W2_d = din("W2", [HD, HD], f32)
    W3_d = din("W3", [HD, HD2], f32)
    b0_d = din("b0c", [P, HD], f32)
    b1_d = din("b1c", [P, HD], f32)
    b2_d = din("b2c", [P, HD], f32)
    b3_d = din("b3c", [P, HD2], f32)
    wsrc_d = din("wsrc_cat", [HD2, 2 * HD], f32)
    wdst_d = din("wdst_cat", [HD2, 2 * HD], f32)
    w2q_d = din("w2q", [P, 8], f32)
    brt_d = din("brt_cat", [1, 2 * HD], f32)
    br2_d = din("br2bt2", [P, 2], f32)
    iota_d = din("iota_f", [P, P], f32)
    identf_d = din("ident_f", [P, P], f32)

    out_d = nc.dram_tensor("out", [P, nck], f32, kind="ExternalOutput")
    if dbg:
        dbg_aug_d = nc.dram_tensor("dbg_aug", [1024, 2 * HD2], bf16,
                                   kind="ExternalOutput")
        dbg_u1_d = nc.dram_tensor("dbg_u1", [1024, HD], mybir.dt.float8e4,
                                  kind="ExternalOutput")
        dbg_h1_d = nc.dram_tensor("dbg_h1", [P, HD], f32,
                                  kind="ExternalOutput")
        dbg_h2_d = nc.dram_tensor("dbg_h2", [P, HD], f32,
                                  kind="ExternalOutput")
        dbg_gs_d = nc.dram_tensor("dbg_gs", [P, 2 * 256], f32,
                                  kind="ExternalOutput")
        dbg_gd_d = nc.dram_tensor("dbg_gd", [P, 2 * 256], f32,
                                  kind="ExternalOutput")
        dbg_dif_d = nc.dram_tensor("dbg_dif", [P, 256], f32,
                                   kind="ExternalOutput")
        dbg_nrm_d = nc.dram_tensor("dbg_nrm", [P, 32], f32,
                                   kind="ExternalOutput")
        dbg_acc_d = nc.dram_tensor("dbg_acc", [P, 64], f32,
                                   kind="ExternalOutput")

    from concourse import library_config
    with tile.TileContext(nc) as tc:
        nc.gpsimd.load_library(library_config.mlp)
        with tc.tile_pool(name="dram", bufs=1, space="DRAM") as dram, \
             tc.tile_pool(name="cpool", bufs=1) as cpool, \
             tc.tile_pool(name="spool", bufs=5) as spool, \
             tc.tile_pool(name="dpool", bufs=2) as dpool:

            # alternate the two SWDGE queues across gathers
            qstate = {"q": 0}

            def next_q():
                qstate["q"] ^= 1
                return qstate["q"]

            # ---------- DRAM intermediates ----------
            def alloc_tables():
                return dict(
                    u1_shard=dram.tile([npc, HD], fp8, name="u1_shard"),
                    u1_full=dram.tile([npad, HD], fp8, name="u1_full",
                                      addr_space="Shared"),
                    u2_shard=dram.tile([npc, HD], fp8, name="u2_shard"),
                    u2_full=dram.tile([npad, HD], fp8, name="u2_full",
                                      addr_space="Shared"),
                    t3_shard=dram.tile([npc, HD2], bf16, name="t3_shard"),
                    t3_full=dram.tile([npad, HD2], bf16, name="t3_full",
                                      addr_space="Shared"),
                    aug_shard=dram.tile([npc, 2 * HD2], bf16,
                                        name="aug_shard"),
                    aug_full=dram.tile([npad, 2 * HD2], bf16, name="aug_full",
                                       addr_space="Shared"),
                )

            # ---------- constants into SBUF ----------
            def load_const(dap, shape, dtype, name):
                t = cpool.tile(list(shape), dtype, name=name)
                nc.sync.dma_start(out=t[:], in_=dap)
                return t

            def load_const_bf(dap, shape, name):
                tf = spool.tile(list(shape), f32, name=name + "_f", tag="cvt")
                nc.sync.dma_start(out=tf[:], in_=dap)
                tb = cpool.tile(list(shape), bf16, name=name)
                nc.scalar.copy(out=tb[:], in_=tf[:])
                return tb

            iota_f32 = spool.tile([P, P], f32, name="iota_f32", tag="cvt")
            nc.sync.dma_start(out=iota_f32[:], in_=iota_d.ap())
            iota_sb = cpool.tile([P, P], bf16, name="iota_sb")
            nc.vector.tensor_copy(out=iota_sb[:], in_=iota_f32[:])
            identf_sb = load_const(identf_d.ap(), [P, P], f32, "identf_sb")
            identb_sb = cpool.tile([P, P], bf16, name="identb_sb")
            nc.vector.tensor_copy(out=identb_sb[:], in_=identf_sb[:])
            b0_sb = load_const(b0_d.ap(), [P, HD], f32, "b0_sb")
            b1_sb = load_const(b1_d.ap(), [P, HD], f32, "b1_sb")
            b2_sb = load_const(b2_d.ap(), [P, HD], f32, "b2_sb")
            b3_sb = load_const(b3_d.ap(), [P, HD2], f32, "b3_sb")
            W0_sb = load_const_bf(W0_d.ap(), [ZD, HD], "W0_sb")
            W1a_sb = load_const_bf(W1_d.ap()[0:P, :], [P, HD], "W1a_sb")
            W1b_sb = load_const_bf(W1_d.ap()[P:HD, :], [P, HD], "W1b_sb")
            W2a_sb = load_const_bf(W2_d.ap()[0:P, :], [P, HD], "W2a_sb")
            W2b_sb = load_const_bf(W2_d.ap()[P:HD, :], [P, HD], "W2b_sb")
            W3a_sb = load_const_bf(W3_d.ap()[0:P, :], [P, HD2], "W3a_sb")
            W3b_sb = load_const_bf(W3_d.ap()[P:HD, :], [P, HD2], "W3b_sb")
            wsrc_sb = load_const_bf(wsrc_d.ap(), [HD2, 2 * HD], "wsrc_sb")
            wdst_sb = load_const_bf(wdst_d.ap(), [HD2, 2 * HD], "wdst_sb")
            w2q_sb = load_const_bf(w2q_d.ap(), [P, 8], "w2q_sb")
            brt_sb = load_const_bf(brt_d.ap(), [1, 2 * HD], "brt_sb")
            br2_sb = load_const(br2_d.ap(), [P, 2], f32, "br2_sb")
            ones2_sb = cpool.tile([1, 3 * P], bf16, name="ones2_sb")
            nc.vector.memset(ones2_sb[:], 1.0)
            onesc_sb = cpool.tile([P, 1], bf16, name="onesc_sb")
            nc.vector.memset(onesc_sb[:], 1.0)

            dcl_sb = load_const(dcl_d.ap(), [P, cl], f32, "dcl_sb")
            dch_sb = load_const(dch_d.ap(), [P, ch], f32, "dch_sb")
            dvl_sb = load_const(dvl_d.ap(), [P, cl], f32, "dvl_sb")
            dvh_sb = load_const(dvh_d.ap(), [P, ch], f32, "dvh_sb")
            glo_sb = load_const(glo_d.ap(), [P, cl * 8], i16, "glo_sb")
            ghi_sb = load_const(ghi_d.ap(), [P, ch * 8], i16, "ghi_sb")

            combo_base = [0]
            for k in range(3):
                combo_base.append(combo_base[-1] + ecs[k])

            for _rep in range(reps):
                _t = alloc_tables()
                u1_shard, u1_full = _t["u1_shard"], _t["u1_full"]
                u2_shard, u2_full = _t["u2_shard"], _t["u2_full"]
                t3_shard, t3_full = _t["t3_shard"], _t["t3_full"]
                aug_shard, aug_full = _t["aug_shard"], _t["aug_full"]
                # ---------- GCN phase ----------
                with tc.tile_pool(name="hpool", bufs=2) as hpool, \
                     tc.tile_pool(name="gpool", bufs=3) as gpool, \
                     tc.tile_pool(name="ohpool", bufs=16) as ohpool, \
                     tc.tile_pool(name="ppool", bufs=4, space="PSUM") as ppool, \
                     tc.tile_pool(name="psum", bufs=2, space="PSUM") as psum:

                    # stage the local z rows into aug[:, HD2:] early (pure
                    # DMA; off the critical path).
                    zstage = hpool.tile([P, nb, ZD], bf16, name="zstage",
                                        tag="zst")
                    for b in range(nb):
                        nc.sync.dma_start(
                            out=zstage[:, b, :],
                            in_=zsh_d.ap()[b * P:(b + 1) * P, :])
                        nc.sync.dma_start(
                            out=aug_shard[b * P:(b + 1) * P, HD2:2 * HD2],
                            in_=zstage[:, b, :])

                    h1_sb = hpool.tile([P, nb, HD], bf16, name="h1_sb", tag="h")
                    h2_sb = hpool.tile([P, nb, HD], bf16, name="h2_sb", tag="h")
                    h3_sb = hpool.tile([P, nb, HD], bf16, name="h3_sb", tag="h")

                    def propagate(table, width, dtype, epilogue, gbw=None):
                        gbw = gb if gbw is None else gbw
                        gclmax = max(off_l[b0_ + gbw] - off_l[b0_]
                                     for b0_ in range(0, nb, gbw))
                        gchmax = max(off_h[b0_ + gbw] - off_h[b0_]
                                     for b0_ in range(0, nb, gbw))
                        tlo = table[0:half, :]
                        thi = table[half:npad, :]
                        for grp in range(nb // gbw):
                            b0 = grp * gbw
                            gcl = off_l[b0 + gbw] - off_l[b0]
                            gch = off_h[b0 + gbw] - off_h[b0]
                            glo = gpool.tile([P, gclmax, width], dtype,
                                             name="glo", tag="glo")
                            nc.gpsimd.dma_gather(
                                out_ap=glo[:, 0:gcl, :], in_ap=tlo,
                                idxs_ap=glo_sb[:, off_l[b0] * 8:
                                               (off_l[b0] + gcl) * 8],
                                num_idxs=gcl * P, num_idxs_reg=gcl * P,
                                elem_size=width, single_packet=False,
                                queue_num=next_q())
                            ghi_t = gpool.tile([P, gchmax, width], dtype,
                                               name="ghi_t", tag="ghi")
                            nc.gpsimd.dma_gather(
                                out_ap=ghi_t[:, 0:gch, :], in_ap=thi,
                                idxs_ap=ghi_sb[:, off_h[b0] * 8:
                                               (off_h[b0] + gch) * 8],
                                num_idxs=gch * P, num_idxs_reg=gch * P,
                                elem_size=width, single_packet=False,
                                queue_num=next_q())
                            if "pc" in skip:
                                continue
                            for bb in range(gbw):
                                b = b0 + bb
                                ncl, nch = cl_b[b], ch_b[b]
                                total = ncl + nch
                                ps = ppool.tile([P, width], f32, name="prop_ps",
                                               tag="prop")
                                idx = 0
                                for j in range(ncl):
                                    col = off_l[b] + j
                                    oh = ohpool.tile([P, P], bf16, name="oh",
                                                     tag="oh")
                                    nc.vector.tensor_scalar(
                                        out=oh[:], in0=iota_sb[:],
                                        scalar1=dcl_sb[:, col:col + 1],
                                        scalar2=dvl_sb[:, col:col + 1],
                                        op0=OP.is_equal, op1=OP.mult)
                                    nc.tensor.matmul(
                                        ps[:], lhsT=oh[:],
                                        rhs=glo[:, off_l[b] - off_l[b0] + j, :],
                                        start=(idx == 0),
                                        stop=(idx == total - 1))
                                    idx += 1
                                for j in range(nch):
                                    col = off_h[b] + j
                                    oh = ohpool.tile([P, P], bf16, name="oh",
                                                     tag="oh")
                                    nc.vector.tensor_scalar(
                                        out=oh[:], in0=iota_sb[:],
                                        scalar1=dch_sb[:, col:col + 1],
                                        scalar2=dvh_sb[:, col:col + 1],
                                        op0=OP.is_equal, op1=OP.mult)
                                    nc.tensor.matmul(
                                        ps[:], lhsT=oh[:],
                                        rhs=ghi_t[:, off_h[b] - off_h[b0] + j, :],
                                        start=(idx == 0),
                                        stop=(idx == total - 1))
                                    idx += 1
                                epilogue(b, ps)

                    def transform_block(h_sb, b, wts, outw, dest,
                                        dest_dtype):
                        ups = psum.tile([P, outw], f32, name="ups", tag="mm")
                        nkh = len(wts)
                        for kh in range(nkh):
                            ht_ps = psum.tile([P, P], bf16, name="ht_ps",
                                              tag="tp")
                            nc.tensor.transpose(
                                ht_ps[:], h_sb[:, b, kh * P:(kh + 1) * P],
                                identb_sb[:])
                            ht = spool.tile([P, P], bf16, name="ht", tag="ht")
                            nc.vector.tensor_copy(out=ht[:], in_=ht_ps[:])
                            nc.tensor.matmul(ups[:], lhsT=ht[:],
                                             rhs=wts[kh][:],
                                             start=(kh == 0),
                                             stop=(kh == nkh - 1))
                        usb = spool.tile([P, outw], dest_dtype, name="usb",
                                         tag="usb")
                        nc.scalar.copy(out=usb[:], in_=ups[:])
                        nc.sync.dma_start(out=dest[b * P:(b + 1) * P, :],
                                          in_=usb[:])

                    def epi0(b, ps):
                        s0 = spool.tile([P, ZD], bf16, name="s0", tag="s0")
                        nc.scalar.copy(out=s0[:], in_=ps[:])
                        s0t_ps = psum.tile([P, P], bf16, name="s0t_ps", tag="tp")
                        nc.tensor.transpose(s0t_ps[:], s0[:], identb_sb[:])
                        s0t = spool.tile([P, P], bf16, name="s0t", tag="s0t")
                        nc.vector.tensor_copy(out=s0t[:], in_=s0t_ps[:])
                        hps = psum.tile([P, HD], f32, name="hps", tag="mm")
                        nc.tensor.matmul(hps[:], lhsT=s0t[:], rhs=W0_sb[:],
                                         start=True, stop=True)
                        if not zb_gcn:
                            nc.vector.tensor_tensor(out=hps[:], in0=hps[:],
                                                    in1=b0_sb[:], op=OP.add)
                        nc.scalar.activation(h1_sb[:, b, :], hps[:], AF.Relu)
                        transform_block(h1_sb, b, [W1a_sb, W1b_sb], HD,
                                        u1_shard, fp8)

                    if "gcn" not in skip:
                        propagate(zt_d.ap(), ZD, bf16, epi0)


                    def epi_mid(h_next, bias_sb, twts, toutw, tdest,
                                tdtype):
                        def epi(b, ps):
                            if not zb_gcn:
                                nc.vector.tensor_tensor(out=ps[:], in0=ps[:],
                                                        in1=bias_sb[:],
                                                        op=OP.add)
                            nc.scalar.activation(h_next[:, b, :], ps[:], AF.Relu)
                            transform_block(h_next, b, twts, toutw, tdest,
                                            tdtype)
                        return epi

                    def do_coll(shard, full):
                        if no_coll:
                            nc.sync.dma_start(out=full[0:npc, :], in_=shard[:])
                        else:
                            nc.gpsimd.collective_compute(
                                "AllGather", OP.bypass, replica_groups=rg,
                                ins=[shard[:].opt()], outs=[full[:].opt()])

                    if "gcn" not in skip:
                        do_coll(u1_shard, u1_full)
                        propagate(u1_full, HD, fp8,
                                  epi_mid(h2_sb, b1_sb, [W2a_sb, W2b_sb], HD,
                                          u2_shard, fp8))

                        do_coll(u2_shard, u2_full)
                        propagate(u2_full, HD, fp8,
                                  epi_mid(h3_sb, b2_sb, [W3a_sb, W3b_sb],
                                          HD2, t3_shard, bf16))

                        do_coll(t3_shard, t3_full)

                    def epi3(b, ps):
                        if not zb_gcn:
                            nc.vector.tensor_tensor(out=ps[:], in0=ps[:],
                                                    in1=b3_sb[:], op=OP.add)
                        gb_ = spool.tile([P, HD2], bf16, name="gb_", tag="gb_")
                        nc.scalar.copy(out=gb_[:], in_=ps[:])
                        nc.sync.dma_start(
                            out=aug_shard[b * P:(b + 1) * P, 0:HD2],
                            in_=gb_[:])

                    if "gcn" not in skip:
                        propagate(t3_full, HD2, bf16, epi3)
                    do_coll(aug_shard, aug_full)
                    if dbg and _rep == 0:
                        nc.sync.dma_start(out=dbg_aug_d.ap(),
                                          in_=aug_full[0:1024, :])
                        nc.sync.dma_start(out=dbg_u1_d.ap(),
                                          in_=u1_full[0:1024, :])
                        dh1 = spool.tile([P, HD], f32, name="dh1", tag="cvt")
                        nc.vector.tensor_copy(out=dh1[:], in_=h1_sb[:, 0, :])
                        nc.sync.dma_start(out=dbg_h1_d.ap(), in_=dh1[:])
                        dh2 = spool.tile([P, HD], f32, name="dh2", tag="cvt")
                        nc.vector.tensor_copy(out=dh2[:], in_=h2_sb[:, 0, :])
                        nc.sync.dma_start(out=dbg_h2_d.ap(), in_=dh2[:])

                if "edge" in skip:
                    nc.sync.dma_start(
                        out=out_d.ap()[:, :],
                        in_=glo_sb[:, 0:2 * nck].bitcast(f32))
                    continue
                # ---------- edge stage ----------
                with tc.tile_pool(name="epool", bufs=3) as epool, \
                     tc.tile_pool(name="fpool", bufs=3) as fpool, \
                     tc.tile_pool(name="jpool", bufs=6) as jpool, \
                     tc.tile_pool(name="idxp", bufs=1) as idxp, \
                     tc.tile_pool(name="hps_pool", bufs=2, space="PSUM") as hps_pool, \
                     tc.tile_pool(name="dps", bufs=1, space="PSUM") as dps, \
                     tc.tile_pool(name="tpp", bufs=2, space="PSUM") as tpp, \
                     tc.tile_pool(name="rt_pool", bufs=1, space="PSUM") as rt_pool:

                    esrc_sb = idxp.tile([P, nck * 8], i16, name="esrc_sb")
                    nc.sync.dma_start(out=esrc_sb[:], in_=esrc_d.ap())
                    edst_sb = idxp.tile([P, nck * 8], i16, name="edst_sb")
                    nc.sync.dma_start(out=edst_sb[:], in_=edst_d.ap())

                    alo_t = aug_full[0:half, :]
                    ahi_t = aug_full[half:npad, :]

                    for k in range(4):
                        s_a = ahi_t if k >= 2 else alo_t
                        d_a = ahi_t if (k % 2) else alo_t
                        nchunks = ecs[k]
                        for c0 in range(0, nchunks, eb2):
                            nbch = min(eb2, nchunks - c0)
                            base = combo_base[k] + c0
                            idx_s = esrc_sb[:, base * 8:(base + nbch) * 8]
                            idx_d = edst_sb[:, base * 8:(base + nbch) * 8]

                            def egather(tab, idxs, name):
                                t = epool.tile([P, 2, nbch * P], bf16,
                                               name=name, tag=name)
                                nc.gpsimd.dma_gather(
                                    out_ap=t[:], in_ap=tab,
                                    idxs_ap=idxs,
                                    num_idxs=nbch * P, num_idxs_reg=nbch * P,
                                    elem_size=2 * HD2, transpose=True,
                                    single_packet=False, queue_num=0)
                                return t

                            gs = egather(s_a, idx_s, "gs")
                            # dst endpoint: edge-major full-row gather on q1
                            # (non-transpose; safe concurrently with the q0
                            # transpose gather), PE-transposed per chunk.
                            gdr = epool.tile([P, nbch, 2 * HD2], bf16,
                                             name="gdr", tag="gdr")
                            nc.gpsimd.dma_gather(
                                out_ap=gdr[:], in_ap=d_a,
                                idxs_ap=idx_d,
                                num_idxs=nbch * P, num_idxs_reg=nbch * P,
                                elem_size=2 * HD2, single_packet=False,
                                queue_num=1)

                            ps_d = dps.tile([P, eb2], f32, name="ps_d",
                                            tag="ps_d")
                            ps_rt = rt_pool.tile([P, eb2, 2], f32, name="ps_rt",
                                                 tag="ps_rt")
                            for cc0 in range(0, nbch, 2):
                                w = min(2, nbch - cc0)
                                cols = w * P
                                # transpose dst g|z halves to [feat, edge]
                                gdt2 = jpool.tile([P, 2, 2 * P], bf16,
                                                  name="gdt2", tag="gdt2")
                                for i in range(w):
                                    for hh in range(2):
                                        etp = tpp.tile([P, P], bf16,
                                                       name="etp", tag="etp")
                                        nc.tensor.transpose(
                                            etp[:],
                                            gdr[:, cc0 + i,
                                                hh * P:(hh + 1) * P],
                                            identb_sb[:])
                                        nc.vector.tensor_copy(
                                            out=gdt2[:, hh, i * P:(i + 1) * P],
                                            in_=etp[:])
                                diffp = fpool.tile([P, 2 * P], bf16,
                                                   name="diffp", tag="diffp")
                                nc.vector.tensor_tensor(
                                    out=diffp[:, 0:cols],
                                    in0=gs[:, 1, cc0 * P:cc0 * P + cols],
                                    in1=gdt2[:, 1, 0:cols],
                                    op=OP.subtract)
                                sqp = fpool.tile([P, 2 * P], bf16, name="sqp",
                                                 tag="sqp")
                                nc.vector.tensor_tensor(
                                    out=sqp[:, 0:cols], in0=diffp[:, 0:cols],
                                    in1=diffp[:, 0:cols], op=OP.mult)
                                for i in range(w):
                                    cc = cc0 + i
                                    nc.tensor.matmul(
                                        ps_d[:, cc:cc + 1],
                                        lhsT=sqp[:, i * P:(i + 1) * P],
                                        rhs=onesc_sb[:], start=True, stop=True)
                                ps_h = hps_pool.tile([P, 4, 2 * P], f32,
                                                     name="ps_h", tag="ps_h")
                                for q in range(4):
                                    if not zb_edge:
                                        nc.tensor.matmul(
                                            ps_h[:, q, 0:cols],
                                            lhsT=brt_sb[:, q * P:(q + 1) * P],
                                            rhs=ones2_sb[:, 0:cols],
                                            start=True, stop=False)
                                    nc.tensor.matmul(
                                        ps_h[:, q, 0:cols],
                                        lhsT=wsrc_sb[:, q * P:(q + 1) * P],
                                        rhs=gs[:, 0, cc0 * P:cc0 * P + cols],
                                        start=zb_edge, stop=False)
                                    nc.tensor.matmul(
                                        ps_h[:, q, 0:cols],
                                        lhsT=wdst_sb[:, q * P:(q + 1) * P],
                                        rhs=gdt2[:, 0, 0:cols],
                                        start=False, stop=True)
                                hact = jpool.tile([P, 4, 2 * P], bf16,
                                                  name="hact", tag="hact")
                                nc.scalar.activation(hact[:, :, 0:cols],
                                                     ps_h[:, :, 0:cols],
                                                     AF.Lrelu, alpha=0.2)
                                for i in range(w):
                                    cc = cc0 + i
                                    for q in range(4):
                                        nc.tensor.matmul(
                                            ps_rt[:, cc, 0:2],
                                            lhsT=hact[:, q, i * P:(i + 1) * P],
                                            rhs=w2q_sb[:, 2 * q:2 * q + 2],
                                            start=(q == 0), stop=(q == 3))
                            nrmb = fpool.tile([P, eb2], f32,
                                              name="nrmb", tag="nrmb")
                            nc.scalar.sqrt(out=nrmb[:, 0:nbch],
                                           in_=ps_d[:, 0:nbch])
                            # finalize batch
                            acc = fpool.tile([P, eb2, 2], f32, name="acc",
                                             tag="acc")
                            nc.vector.tensor_copy(out=acc[:, 0:nbch, :],
                                                  in_=ps_rt[:, 0:nbch, :])
                            tt = fpool.tile([P, eb2], f32, name="tt", tag="tt")
                            nc.vector.tensor_scalar(
                                out=tt[:, 0:nbch],
                                in0=acc[:, 0:nbch, 1],
                                scalar1=br2_sb[:, 1:2], scalar2=None,
                                op0=OP.add)
                            tinv = fpool.tile([P, eb2], f32, name="tinv",
                                              tag="tinv")
                            nc.vector.reciprocal(out=tinv[:, 0:nbch],
                                                 in_=tt[:, 0:nbch])
                            num = fpool.tile([P, eb2], f32, name="num", tag="num")
                            nc.vector.tensor_tensor(out=num[:, 0:nbch],
                                                    in0=nrmb[:, 0:nbch],
                                                    in1=acc[:, 0:nbch, 0],
                                                    op=OP.add)
                            if not zb_br2:
                                nc.vector.tensor_scalar(
                                    out=num[:, 0:nbch], in0=num[:, 0:nbch],
                                    scalar1=br2_sb[:, 0:1], scalar2=None,
                                    op0=OP.add)
                            xx = fpool.tile([P, eb2], f32, name="xx", tag="xx")
                            nc.vector.tensor_tensor(out=xx[:, 0:nbch],
                                                    in0=num[:, 0:nbch],
                                                    in1=tinv[:, 0:nbch],
                                                    op=OP.mult)
                            th = fpool.tile([P, eb2], f32, name="th", tag="th")
                            nc.scalar.activation(th[:, 0:nbch], xx[:, 0:nbch],
                                                 AF.Tanh, scale=-0.5)
                            osb = fpool.tile([P, eb2], f32, name="osb", tag="osb")
                            nc.vector.tensor_scalar(
                                out=osb[:, 0:nbch], in0=th[:, 0:nbch],
                                scalar1=0.5, scalar2=0.5,
                                op0=OP.mult, op1=OP.add)
                            nc.sync.dma_start(
                                out=out_d.ap()[:, base:base + nbch],
                                in_=osb[:, 0:nbch])
    nc.finalize()
    return nc


# --------------------------------------------------------------------------
# Input staging
# --------------------------------------------------------------------------

def stage_inputs(meta, percore, host, inputs):
    npc, nb = meta["npc"], meta["nb"]
    ncores = meta["ncores"]
    old2new = host["old2new"]
    z = np.asarray(inputs["z"], np.float32)

    zpad = np.zeros((meta["npad"], ZD), np.float32)
    zpad[old2new] = z
    import ml_dtypes
    zt = np.ascontiguousarray(zpad.astype(ml_dtypes.bfloat16))

    def bc(v, w):
        v = np.asarray(v, np.float32).reshape(-1)
        return np.ascontiguousarray(np.broadcast_to(v, (P, w)))

    Wr1 = np.asarray(inputs["Wr1"], np.float32)
    Wt1 = np.asarray(inputs["Wt1"], np.float32)
    wsrc = np.ascontiguousarray(
        np.concatenate([Wr1[:HD2], Wt1[:HD2]], axis=1))
    wdst = np.ascontiguousarray(
        np.concatenate([Wr1[HD2:], Wt1[HD2:]], axis=1))
    wr2 = np.asarray(inputs["Wr2"], np.float32)[:, 0]
    wt2 = np.asarray(inputs["Wt2"], np.float32)[:, 0]
    w2q = np.zeros((P, 4, 2), np.float32)
    w2q[:, 0, 0] = wr2[0:P]
    w2q[:, 1, 0] = wr2[P:HD]
    w2q[:, 2, 1] = wt2[0:P]
    w2q[:, 3, 1] = wt2[P:HD]
    brt = np.ascontiguousarray(np.concatenate(
        [np.asarray(inputs["br1"], np.float32),
         np.asarray(inputs["bt1"], np.float32)])[None, :])
    br2v = np.array([[float(np.asarray(inputs["br2"]).reshape(-1)[0]),
                      float(np.asarray(inputs["bt2"]).reshape(-1)[0])]],
                    np.float32)
    iota = np.ascontiguousarray(
        np.broadcast_to(np.arange(P, dtype=np.float32)[None, :], (P, P)))

    in_maps = []
    for c in range(ncores):
        pc = percore[c]
        m = {
            "z_tbl": zt,
            "z_shard": np.ascontiguousarray(zt[c * npc:(c + 1) * npc]),
            "gidx_lo": pc["gidx_lo"], "gidx_hi": pc["gidx_hi"],
            "dcol_lo": pc["dcol_lo"], "dcol_hi": pc["dcol_hi"],
            "dval_lo": pc["dval_lo"], "dval_hi": pc["dval_hi"],
            "eidx_src": pc["eidx_src"], "eidx_dst": pc["eidx_dst"],
            "W0": np.asarray(inputs["W0"], np.float32),
            "W1": np.asarray(inputs["W1"], np.float32),
            "W2": np.asarray(inputs["W2"], np.float32),
            "W3": np.asarray(inputs["W3"], np.float32),
            "b0c": bc(inputs["b0"], HD), "b1c": bc(inputs["b1"], HD),
            "b2c": bc(inputs["b2"], HD), "b3c": bc(inputs["b3"], HD2),
            "wsrc_cat": wsrc, "wdst_cat": wdst,
            "w2q": np.ascontiguousarray(w2q.reshape(P, 8)),
            "brt_cat": brt,
            "br2bt2": np.ascontiguousarray(np.broadcast_to(br2v, (P, 2))),
            "iota_f": iota,
            "ident_f": np.eye(P, dtype=np.float32),
        }
        in_maps.append(m)
    return in_maps


def assemble_output(meta, host, results, E):
    out = np.zeros(E, np.float32)
    slotmap = host["slotmap"]
    for c in range(meta["ncores"]):
        buf = np.asarray(results[c]["out"]).astype(np.float32)  # [P, nck]
        vals = buf.T.reshape(-1)                   # slot = chunk*P + p
        sm = slotmap[c]
        ok = sm >= 0
        out[sm[ok]] = vals[ok]
    return out


# --------------------------------------------------------------------------
# Entry point
# --------------------------------------------------------------------------

_CACHE = {}


def kernel(**inputs):
    edge_index = np.asarray(inputs["edge_index"])
    N = np.asarray(inputs["z"]).shape[0]
    E = edge_index.shape[1]

    meta, percore, host = build_plan(edge_index, N)
    zb_gcn = all(
        not np.any(np.asarray(inputs[k])) for k in ("b0", "b1", "b2", "b3"))
    zb_edge = not (np.any(np.asarray(inputs["br1"]))
                   or np.any(np.asarray(inputs["bt1"])))
    zb_br2 = not np.any(np.asarray(inputs["br2"]))
    meta["zbias"] = (zb_gcn, zb_edge, zb_br2)
    key = tuple(sorted((k, v) for k, v in meta.items()))
    if key not in _CACHE:
        _CACHE[key] = build_nc(meta, debug=False)
    nc = _CACHE[key]

    in_maps = stage_inputs(meta, percore, host, inputs)
    from concourse.bass_utils import run_bass_kernel_spmd
    import os
    trace = bool(int(os.environ.get("KERNEL_TRACE", "0")))
    res = run_bass_kernel_spmd(nc, in_maps,
                               core_ids=list(range(meta["ncores"])),
                               trace=trace)
    kernel._last_res = res
    return assemble_output(meta, host, res.results, E)
